# revision 13
# baseline (speedup 1.0000x reference)
"""ConvLSTM cell kernel for Trainium2 (8 NeuronCores, data-parallel over batch).

Problem: SEQ=32 steps of x_t (B,64,256); per step:
  gates = conv1d(concat([x_t, h], ch), W, k=5, pad=2) + b   -> (B,512,256)
  gates = GroupNorm(16 groups, eps=1e-5) * gn_w + gn_b
  i,f,g,o = split(gates); c = sig(f)*c + sig(i)*tanh(g); h = sig(o)*tanh(c)
Returns (outputs[SEQ,B,128,256], (h, c)).

Design (per core, 4 batch elems):
- conv as 8 accumulating PE matmuls per (gate, half-batch): K-dim packs two
  x-shifts per matmul via a double-loaded x tile with a 1-column offset
  between row halves; h-part is 5 full-K matmuls over a shared padded tile.
  All matmuls in float32r (full PE rate at N=512).
- GroupNorm stats: bn_stats per PSUM gate tile; cross-partition (32-ch group)
  reduction via a 3-op 32x32 transpose-reduce (tensor_reduce apply_transpose,
  broadcast, stream-transpose back); rsqrt via Quake-seed + 2 Newton
  iterations on DVE; result folded with conv bias, gn_w/gn_b into
  per-partition scale/bias consumed directly by the fused ScalarE
  sigmoid/tanh activation reading PSUM. Stats run as two pipelines:
  gates {i,f,g} overlap the o-gate matmul phase; only the o-gate chain sits
  on the recurrent tail, and the final h'=sig(o)*tanh(c') multiply is
  emitted per batch element to pipeline with the o-gate activations.
"""
import sys

sys.path.insert(0, "/opt/trn_rl_repo")

from contextlib import ExitStack

import numpy as np

import concourse.bass as bass
import concourse.tile as tile
from concourse import mybir
from concourse.bass_utils import run_bass_kernel_spmd

SEQ, B, F, L = 32, 32, 64, 256
OC = 128
NCORES = 8
BC = B // NCORES      # batch per core = 4
SEG = L + 4           # padded segment width = 260
EPS = 1e-5

f32 = mybir.dt.float32
f32r = mybir.dt.float32r
i32 = mybir.dt.int32
Alu = mybir.AluOpType
Act = mybir.ActivationFunctionType

GATE_ORDER = [0, 1, 2, 3]       # i, f, g, o  (o last: shortest recurrent tail)
GATE_FUNC = {0: Act.Sigmoid, 1: Act.Sigmoid, 2: Act.Tanh, 3: Act.Sigmoid}


def _split_multi_waits(nc):
    """This walrus build rejects instructions with >1 sem wait. Split extras
    onto preceding same-engine NoOps."""
    for fn in nc.m.functions:
        for bb in fn.blocks:
            new_insts = []
            for inst in bb.instructions:
                si = inst.sync_info
                waits = list(si.on_wait) if (si is not None and si.on_wait) else []
                if len(waits) > 1:
                    head, rest = waits[:-1], waits[-1:]
                    for j, w in enumerate(head):
                        nop = mybir.InstNoOp(name=f"{inst.name}-wsplit{j}", hint=None)
                        nop.engine = inst.engine
                        nop.sync_info = mybir.SyncInfo(on_wait=[w], on_update=[])
                        nc.register_instruction(nop)
                        new_insts.append(nop)
                    si.on_wait = rest
                new_insts.append(inst)
            bb.instructions = new_insts


def build(T=SEQ, TRIVIAL_GN=False, SPLIT_STATS=True, LSTM_GPSIMD=False,
          NEWTON_ITERS=2, STP_BUFS=2, N1024=False, TRREDUCE=False,
          ABL_HSHIFTS=5, ABL_BFLY=5, ABL_ACT8=False, ABL_NBN=2):
    nc = bass.Bass("TRN2", debug=False)
    x_d = nc.dram_tensor("x", [T, BC, F, L], f32r, kind="ExternalInput").ap()
    wx_d = nc.dram_tensor("wx", [128, 4 * 3 * 128], f32r, kind="ExternalInput").ap()
    wh_d = nc.dram_tensor("wh", [128, 4 * 5 * 128], f32r, kind="ExternalInput").ap()
    cst_d = nc.dram_tensor("cst", [128, 80], f32, kind="ExternalInput").ap()
    zf_d = nc.dram_tensor("zf", [128, BC * SEG], f32r, kind="ExternalInput").ap()
    out_d = nc.dram_tensor("outs", [T, BC, OC, L], f32r, kind="ExternalOutput").ap()
    cy_d = nc.dram_tensor("cy", [BC, OC, L], f32, kind="ExternalOutput").ap()

    with tile.TileContext(nc) as tc, ExitStack() as ctx:
        singles = ctx.enter_context(tc.tile_pool(name="singles", bufs=1))
        gps = ctx.enter_context(
            tc.tile_pool(name="gps", bufs=(4 if N1024 else 8), space="PSUM"))
        stp = ctx.enter_context(tc.tile_pool(name="stp", bufs=STP_BUFS))

        # ---- constants / weights ----
        wx = singles.tile([128, 4 * 3 * 128], f32r)     # lhsT packs: (gate, j, 128)
        wh = singles.tile([128, 4 * 5 * 128], f32r)     # lhsT packs: (gate, k, 128)
        cst = singles.tile([128, 80], f32)
        nc.sync.dma_start(wx[:], wx_d)
        nc.sync.dma_start(wh[:], wh_d)
        nc.sync.dma_start(cst[:], cst_d)
        gw4 = cst[:, 0:16]        # gn_w per (gate, b)
        b4 = cst[:, 16:32]        # conv_b per (gate, b)
        gb4 = cst[:, 32:48]       # gn_b per (gate, b)
        b4h = cst[:, 48:80]       # conv_b per (gate, b, half)

        # ---- state tiles ----
        xt = [singles.tile([128, BC * SEG], f32r, tag=f"xt{p}", name=f"xt{p}")
              for p in range(2)]
        ht = singles.tile([128, BC * SEG], f32r)
        cx = singles.tile([128, BC * L], f32)
        gsb = [singles.tile([128, BC * L], f32, tag=f"gsb{g}", name=f"gsb{g}")
               for g in range(4)]
        tcy = singles.tile([128, BC * L], f32)
        bnall = singles.tile([128, 8, 2, 6], f32)     # (gate*2+hb, b_in_half, sextet)
        ones32 = singles.tile([128, 32], f32)
        nc.vector.memset(ones32[:], 1.0)
        for t_ in xt:
            nc.sync.dma_start(t_[:], zf_d)
        nc.sync.dma_start(ht[:], zf_d)
        nc.vector.memset(cx[:], 0.0)

        def x_dma(t):
            src = x_d[t].rearrange("b c l -> c b l")
            dst = xt[t % 2][:].rearrange("p (s c) -> p s c", s=BC)
            nc.sync.dma_start(dst[0:F, :, 2:2 + L], src)
            nc.sync.dma_start(dst[F:2 * F, :, 1:1 + L], src)

        x_dma(0)

        for t in range(T):
            if t + 1 < T:
                x_dma(t + 1)
            xv = xt[t % 2][:].rearrange("p (s c) -> p s c", s=BC)
            hv = ht[:].rearrange("p (s c) -> p s c", s=BC)

            # ---- PE: 8 accumulating matmuls per (gate, half-batch)
            # (or per gate with N=1024 when N1024) ----
            psums = {}
            nhb = 1 if N1024 else 2
            nseg = BC // nhb
            for g in GATE_ORDER:
                for hb in range(nhb):
                    ps = gps.tile([128, nseg * L], f32, tag="gate_ps")
                    psums[(g, hb)] = ps
                    sl = slice(hb * nseg, (hb + 1) * nseg)
                    for j, off in enumerate((0, 2, 4)):     # x pairs {0,1},{2,3},{4}
                        nc.tensor.matmul(
                            ps[:], wx[:, (g * 3 + j) * 128:(g * 3 + j + 1) * 128],
                            xv[:, sl, off:off + L],
                            start=(j == 0), stop=False)
                    for k in range(ABL_HSHIFTS):             # h shifts
                        nc.tensor.matmul(
                            ps[:], wh[:, (g * 5 + k) * 128:(g * 5 + k + 1) * 128],
                            hv[:, sl, k:k + L],
                            start=False, stop=(k == ABL_HSHIFTS - 1))
                    for bi in range(nseg)[:ABL_NBN]:
                        b = hb * nseg + bi
                        nc.vector.bn_stats(
                            out=bnall[:, g * 2 + b // 2, b % 2, :],
                            in_=ps[:, bi * L:(bi + 1) * L])

            # ---- GroupNorm stats: group A = gates {i,f,g} (overlaps o's
            # matmul phase), group B = {o} (short recurrent tail) ----
            s_sb = stp.tile([128, 16], f32, tag="s_sb")
            t_sb = stp.tile([128, 16], f32, tag="t_sb")

            def stats_chain(g0, ng, suf):
                NC_ = 4 * ng            # combos in this group
                wpad = ((4 * NC_ + 31) // 32) * 32 if TRREDUCE else 4 * NC_
                stage = stp.tile([128, wpad], f32, tag=f"stage{suf}",
                                 name=f"stage{suf}")
                if TRREDUCE:
                    nc.vector.memset(stage[:], 0.0)
                shf = stp.tile([128, wpad], f32, tag=f"shf{suf}",
                               name=f"shf{suf}")
                tmp = stp.tile([128, 2 * NC_], f32, tag=f"tmp{suf}",
                               name=f"tmp{suf}")
                bsl = bnall[:, 2 * g0:2 * (g0 + ng), :, :]
                means = bsl[:, :, :, 1::3]          # [128, 2ng, 2, 2]
                m2s = bsl[:, :, :, 2::3]
                b4h_g = cst[:, 48 + 8 * g0:48 + 8 * (g0 + ng)]
                nc.vector.tensor_tensor(out=stage[:, 0:2 * NC_], in0=means,
                                        in1=b4h_g, op=Alu.add)            # m'
                nc.vector.tensor_tensor(out=tmp[:], in0=stage[:, 0:2 * NC_],
                                        in1=stage[:, 0:2 * NC_], op=Alu.mult)
                nc.vector.scalar_tensor_tensor(
                    out=stage[:, 2 * NC_:4 * NC_], in0=m2s, scalar=1.0 / 128,
                    in1=tmp[:], op0=Alu.mult, op1=Alu.add)                # q
                if TRREDUCE:
                    # cross-partition group sums via 32x32 transpose-reduce:
                    # r[32a+i] = sum_{p in grp a} stage[p, blk*32+i]; then
                    # broadcast r across the block and transpose back.
                    nblk = wpad // 32
                    rsum = stp.tile([128, nblk], f32, tag=f"rsum{suf}",
                                    name=f"rsum{suf}")
                    for blk in range(nblk):
                        nc.vector.tensor_reduce(
                            out=rsum[:, blk:blk + 1],
                            in_=stage[:, blk * 32:(blk + 1) * 32],
                            axis=mybir.AxisListType.X, op=Alu.add,
                            apply_transpose=True)
                        nc.vector.tensor_scalar(
                            out=shf[:, blk * 32:(blk + 1) * 32], in0=ones32[:],
                            scalar1=rsum[:, blk:blk + 1], scalar2=None,
                            op0=Alu.mult)
                        nc.vector.transpose(
                            out=stage[:, blk * 32:(blk + 1) * 32],
                            in_=shf[:, blk * 32:(blk + 1) * 32])
                else:
                    for r in (16, 8, 4, 2, 1)[:ABL_BFLY]:
                        nc.vector.stream_shuffle(shf[:], stage[:],
                                                 [i ^ r for i in range(32)])
                        nc.vector.tensor_tensor(out=stage[:], in0=stage[:],
                                                in1=shf[:], op=Alu.add)
                sums = stp.tile([128, 2 * NC_], f32, tag=f"sums{suf}",
                                name=f"sums{suf}")
                st3 = stage[:, 0:4 * NC_].rearrange("p (c h) -> p c h", h=2)
                nc.vector.tensor_tensor(out=sums[:], in0=st3[:, :, 0],
                                        in1=st3[:, :, 1], op=Alu.add)
                s_m, s_q = sums[:, 0:NC_], sums[:, NC_:2 * NC_]
                csl = slice(4 * g0, 4 * (g0 + ng))
                # veps = S_q/64 - S_m^2/4096 + eps
                t2 = stp.tile([128, NC_], f32, tag=f"t2{suf}", name=f"t2{suf}")
                nc.vector.tensor_tensor(out=t2[:], in0=s_m, in1=s_m, op=Alu.mult)
                nc.vector.tensor_scalar(out=t2[:], in0=t2[:], scalar1=-1.0 / 4096,
                                        scalar2=EPS, op0=Alu.mult, op1=Alu.add)
                veps = stp.tile([128, NC_], f32, tag=f"veps{suf}",
                                name=f"veps{suf}")
                nc.vector.scalar_tensor_tensor(out=veps[:], in0=s_q,
                                               scalar=1.0 / 64, in1=t2[:],
                                               op0=Alu.mult, op1=Alu.add)
                # rs = rsqrt(veps): Quake seed + 2 Newton iterations
                rs = s_sb[:, csl]
                un = stp.tile([128, NC_], f32, tag=f"un{suf}", name=f"un{suf}")
                nc.vector.tensor_scalar(out=rs.bitcast(i32),
                                        in0=veps[:].bitcast(i32), scalar1=1,
                                        scalar2=None, op0=Alu.logical_shift_right)
                nc.vector.tensor_scalar(out=rs.bitcast(i32), in0=rs.bitcast(i32),
                                        scalar1=-1, scalar2=None,
                                        op0=Alu.bitwise_xor)
                nc.vector.tensor_scalar(out=rs.bitcast(i32), in0=rs.bitcast(i32),
                                        scalar1=0x5F3759E0, scalar2=None,
                                        op0=Alu.add)
                for _ in range(NEWTON_ITERS):
                    nc.vector.tensor_tensor(out=un[:], in0=veps[:], in1=rs,
                                            op=Alu.mult)
                    nc.vector.tensor_tensor(out=un[:], in0=un[:], in1=rs,
                                            op=Alu.mult)
                    nc.vector.tensor_scalar(out=un[:], in0=un[:], scalar1=-0.5,
                                            scalar2=1.5, op0=Alu.mult,
                                            op1=Alu.add)
                    nc.vector.tensor_tensor(out=rs, in0=rs, in1=un[:],
                                            op=Alu.mult)
                # s = rs*gw (skipped when gn_w==1) ; t = (cb - S_m/64)*s + gb
                if not TRIVIAL_GN:
                    nc.vector.tensor_tensor(out=rs, in0=rs, in1=cst[:, csl],
                                            op=Alu.mult)
                tg = t_sb[:, csl]
                nc.vector.scalar_tensor_tensor(out=tg, in0=s_m, scalar=-1.0 / 64,
                                               in1=cst[:, 16:32][:, csl],
                                               op0=Alu.mult, op1=Alu.add)
                nc.vector.tensor_tensor(out=tg, in0=tg, in1=rs, op=Alu.mult)
                if not TRIVIAL_GN:
                    nc.vector.tensor_tensor(out=tg, in0=tg,
                                            in1=cst[:, 32:48][:, csl], op=Alu.add)

            if SPLIT_STATS:
                stats_chain(0, 3, "A")
                stats_chain(3, 1, "B")
            else:
                stats_chain(0, 4, "A")

            # ---- fused normalize + nonlinearity (ScalarE reads PSUM) ----
            for g in GATE_ORDER:
                for b in range(1 if ABL_ACT8 else BC):
                    c = 4 * g + b
                    nc.scalar.activation(
                        out=gsb[g][:, b * L:(b + 1) * L] if not ABL_ACT8
                        else gsb[g][:],
                        in_=(psums[(g, 0)][:, b * L:(b + 1) * L] if N1024 else
                             (psums[(g, b // 2)][:, (b % 2) * L:(b % 2 + 1) * L]
                              if not ABL_ACT8 else psums[(g, 0)][:])),
                        func=GATE_FUNC[g],
                        bias=t_sb[:, c:c + 1], scale=s_sb[:, c:c + 1])

            # ---- LSTM combine ----
            eng = nc.gpsimd if LSTM_GPSIMD else nc.vector
            eng.tensor_tensor(out=gsb[2][:], in0=gsb[0][:], in1=gsb[2][:],
                              op=Alu.mult)                          # i*g
            eng.tensor_tensor(out=gsb[1][:], in0=gsb[1][:], in1=cx[:],
                              op=Alu.mult)                          # f*c
            eng.tensor_tensor(out=cx[:], in0=gsb[1][:], in1=gsb[2][:],
                              op=Alu.add)                           # c'
            nc.scalar.activation(out=tcy[:], in_=cx[:], func=Act.Tanh)
            hv_data = hv[:, :, 2:2 + L]
            for b in range(BC):                     # h' = sig(o)*tanh(c'), per b
                nc.vector.tensor_tensor(            # pipelines with ACT o calls
                    out=hv_data[:, b, :],
                    in0=gsb[3][:, b * L:(b + 1) * L],
                    in1=tcy[:, b * L:(b + 1) * L],
                    op=Alu.mult)
            nc.sync.dma_start(out_d[t].rearrange("b p l -> p b l"), hv_data)

        nc.sync.dma_start(cy_d.rearrange("b p l -> p b l"),
                          cx[:].rearrange("p (b l) -> p b l", b=BC))

    _split_multi_waits(nc)
    return nc


def prep_shared(conv_w, conv_b, gn_w, gn_b):
    """Host packing of weights/constants (shared across cores)."""
    W = np.asarray(conv_w, dtype=np.float32)        # (512, 192, 5)
    cb = np.asarray(conv_b, dtype=np.float32)
    gw = np.asarray(gn_w, dtype=np.float32)
    gb = np.asarray(gn_b, dtype=np.float32)

    wx = np.zeros((128, 4, 3, 128), dtype=np.float32)
    wh = np.zeros((128, 4, 5, 128), dtype=np.float32)
    for g in range(4):
        Wg = W[g * 128:(g + 1) * 128]               # (128, 192, 5)
        Wxg = Wg[:, :F, :]                          # (128, 64, 5)
        Whg = Wg[:, F:, :]                          # (128, 128, 5)
        wx[0:64, g, 0, :] = Wxg[:, :, 0].T
        wx[64:128, g, 0, :] = Wxg[:, :, 1].T
        wx[0:64, g, 1, :] = Wxg[:, :, 2].T
        wx[64:128, g, 1, :] = Wxg[:, :, 3].T
        wx[0:64, g, 2, :] = Wxg[:, :, 4].T
        for k in range(5):
            wh[:, g, k, :] = Whg[:, :, k].T

    cst = np.zeros((128, 80), dtype=np.float32)
    for g in range(4):
        for b in range(BC):
            cst[:, 0 + 4 * g + b] = gw[g * 128:(g + 1) * 128]
            cst[:, 16 + 4 * g + b] = cb[g * 128:(g + 1) * 128]
            cst[:, 32 + 4 * g + b] = gb[g * 128:(g + 1) * 128]
            for h in range(2):
                cst[:, 48 + (4 * g + b) * 2 + h] = cb[g * 128:(g + 1) * 128]
    return (wx.reshape(128, -1), wh.reshape(128, -1), cst)


_CACHE = {}


def kernel(inputs, conv_w, conv_b, gn_w, gn_b):
    x = np.asarray(inputs, dtype=np.float32)        # (32, 32, 64, 256)
    wx, wh, cst = prep_shared(conv_w, conv_b, gn_w, gn_b)

    trivial = bool(np.all(np.asarray(gn_w) == 1.0) and np.all(np.asarray(gn_b) == 0.0))
    key = ("nc", trivial)
    if key not in _CACHE:
        _CACHE[key] = build(SEQ, TRIVIAL_GN=trivial, TRREDUCE=True)
    nc = _CACHE[key]

    in_maps = []
    for c in range(NCORES):
        shard = np.ascontiguousarray(x[:, c * BC:(c + 1) * BC])
        in_maps.append({"x": shard, "wx": wx, "wh": wh, "cst": cst,
                        "zf": np.zeros((128, BC * SEG), np.float32)})

    res = run_bass_kernel_spmd(nc, in_maps, list(range(NCORES)))

    outputs = np.concatenate([res.results[c]["outs"] for c in range(NCORES)], axis=1)
    cy = np.concatenate([res.results[c]["cy"] for c in range(NCORES)], axis=0)
    hy = outputs[-1]
    return outputs, (hy, cy)


# revision 15
# speedup vs baseline: 1.0141x; 1.0141x over previous
"""ConvLSTM cell kernel for Trainium2 (8 NeuronCores, data-parallel over batch).

Problem: SEQ=32 steps of x_t (B,64,256); per step:
  gates = conv1d(concat([x_t, h], ch), W, k=5, pad=2) + b   -> (B,512,256)
  gates = GroupNorm(16 groups, eps=1e-5) * gn_w + gn_b
  i,f,g,o = split(gates); c = sig(f)*c + sig(i)*tanh(g); h = sig(o)*tanh(c)
Returns (outputs[SEQ,B,128,256], (h, c)).

Design (per core, 4 batch elems):
- conv as 8 accumulating PE matmuls per (gate, half-batch): K-dim packs two
  x-shifts per matmul via a double-loaded x tile with a 1-column offset
  between row halves; h-part is 5 full-K matmuls over a shared padded tile.
  All matmuls in float32r (full PE rate at N=512).
- GroupNorm stats: bn_stats per PSUM gate tile; cross-partition (32-ch group)
  reduction via a 3-op 32x32 transpose-reduce (tensor_reduce apply_transpose,
  broadcast, stream-transpose back); rsqrt via Quake-seed + 2 Newton
  iterations on DVE; result folded with conv bias, gn_w/gn_b into
  per-partition scale/bias consumed directly by the fused ScalarE
  sigmoid/tanh activation reading PSUM. Stats run as two pipelines:
  gates {i,f,g} overlap the o-gate matmul phase; only the o-gate chain sits
  on the recurrent tail, and the final h'=sig(o)*tanh(c') multiply is
  emitted per batch element to pipeline with the o-gate activations.
"""
import sys

sys.path.insert(0, "/opt/trn_rl_repo")

from contextlib import ExitStack

import numpy as np

import concourse.bass as bass
import concourse.tile as tile
from concourse import mybir
from concourse.bass_utils import run_bass_kernel_spmd

SEQ, B, F, L = 32, 32, 64, 256
OC = 128
NCORES = 8
BC = B // NCORES      # batch per core = 4
SEG = L + 4           # padded segment width = 260
EPS = 1e-5

f32 = mybir.dt.float32
f32r = mybir.dt.float32r
i32 = mybir.dt.int32
Alu = mybir.AluOpType
Act = mybir.ActivationFunctionType

GATE_ORDER = [0, 1, 2, 3]       # i, f, g, o  (o last: shortest recurrent tail)
GATE_FUNC = {0: Act.Sigmoid, 1: Act.Sigmoid, 2: Act.Tanh, 3: Act.Sigmoid}


def _split_multi_waits(nc):
    """This walrus build rejects instructions with >1 sem wait. Split extras
    onto preceding same-engine NoOps."""
    for fn in nc.m.functions:
        for bb in fn.blocks:
            new_insts = []
            for inst in bb.instructions:
                si = inst.sync_info
                waits = list(si.on_wait) if (si is not None and si.on_wait) else []
                if len(waits) > 1:
                    head, rest = waits[:-1], waits[-1:]
                    for j, w in enumerate(head):
                        nop = mybir.InstNoOp(name=f"{inst.name}-wsplit{j}", hint=None)
                        nop.engine = inst.engine
                        nop.sync_info = mybir.SyncInfo(on_wait=[w], on_update=[])
                        nc.register_instruction(nop)
                        new_insts.append(nop)
                    si.on_wait = rest
                new_insts.append(inst)
            bb.instructions = new_insts


def build(T=SEQ, TRIVIAL_GN=False, SPLIT_STATS=True, LSTM_GPSIMD=False,
          NEWTON_ITERS=2, STP_BUFS=2, N1024=False, TRREDUCE=False, ILV=False,
          ABL_HSHIFTS=5, ABL_BFLY=5, ABL_ACT8=False, ABL_NBN=2):
    nc = bass.Bass("TRN2", debug=False)
    x_d = nc.dram_tensor("x", [T, BC, F, L], f32r, kind="ExternalInput").ap()
    wx_d = nc.dram_tensor("wx", [128, 4 * 3 * 128], f32r, kind="ExternalInput").ap()
    wh_d = nc.dram_tensor("wh", [128, 4 * 5 * 128], f32r, kind="ExternalInput").ap()
    cst_d = nc.dram_tensor("cst", [128, 80], f32, kind="ExternalInput").ap()
    zf_d = nc.dram_tensor("zf", [128, BC * SEG], f32r, kind="ExternalInput").ap()
    out_d = nc.dram_tensor("outs", [T, BC, OC, L], f32r, kind="ExternalOutput").ap()
    cy_d = nc.dram_tensor("cy", [BC, OC, L], f32, kind="ExternalOutput").ap()

    with tile.TileContext(nc) as tc, ExitStack() as ctx:
        singles = ctx.enter_context(tc.tile_pool(name="singles", bufs=1))
        gps = ctx.enter_context(
            tc.tile_pool(name="gps", bufs=(4 if N1024 else 8), space="PSUM"))
        stp = ctx.enter_context(tc.tile_pool(name="stp", bufs=STP_BUFS))

        # ---- constants / weights ----
        wx = singles.tile([128, 4 * 3 * 128], f32r)     # lhsT packs: (gate, j, 128)
        wh = singles.tile([128, 4 * 5 * 128], f32r)     # lhsT packs: (gate, k, 128)
        cst = singles.tile([128, 80], f32)
        nc.sync.dma_start(wx[:], wx_d)
        nc.sync.dma_start(wh[:], wh_d)
        nc.sync.dma_start(cst[:], cst_d)
        gw4 = cst[:, 0:16]        # gn_w per (gate, b)
        b4 = cst[:, 16:32]        # conv_b per (gate, b)
        gb4 = cst[:, 32:48]       # gn_b per (gate, b)
        b4h = cst[:, 48:80]       # conv_b per (gate, b, half)

        # ---- state tiles ----
        xt = [singles.tile([128, BC * SEG], f32r, tag=f"xt{p}", name=f"xt{p}")
              for p in range(2)]
        ht = singles.tile([128, BC * SEG], f32r)
        cx = singles.tile([128, BC * L], f32)
        gsb = [singles.tile([128, BC * L], f32, tag=f"gsb{g}", name=f"gsb{g}")
               for g in range(4)]
        tcy = singles.tile([128, BC * L], f32)
        bnall = singles.tile([128, 8, 2, 6], f32)     # (gate*2+hb, b_in_half, sextet)
        ones32 = singles.tile([128, 32], f32)
        nc.vector.memset(ones32[:], 1.0)
        stages = {}
        if TRREDUCE:
            for suf, ng in (("A", 3 if SPLIT_STATS else 4), ("B", 1)):
                wpad = ((16 * ng + 31) // 32) * 32
                for par in range(2):
                    st_ = singles.tile([128, wpad], f32, tag=f"stg{suf}{par}",
                                       name=f"stg{suf}{par}")
                    nc.vector.memset(st_[:], 0.0)
                    stages[(suf, par)] = st_
        for t_ in xt:
            nc.sync.dma_start(t_[:], zf_d)
        nc.sync.dma_start(ht[:], zf_d)
        nc.vector.memset(cx[:], 0.0)

        def x_dma(t):
            src = x_d[t].rearrange("b c l -> c b l")
            dst = xt[t % 2][:].rearrange("p (s c) -> p s c", s=BC)
            nc.sync.dma_start(dst[0:F, :, 2:2 + L], src)
            nc.sync.dma_start(dst[F:2 * F, :, 1:1 + L], src)

        x_dma(0)

        for t in range(T):
            if t + 1 < T:
                x_dma(t + 1)
            xv = xt[t % 2][:].rearrange("p (s c) -> p s c", s=BC)
            hv = ht[:].rearrange("p (s c) -> p s c", s=BC)

            # ---- PE: 8 accumulating matmuls per (gate, half-batch)
            # (or per gate with N=1024 when N1024) ----
            psums = {}
            nhb = 1 if N1024 else 2
            nseg = BC // nhb
            for g in GATE_ORDER:
                for hb in range(nhb):
                    ps = gps.tile([128, nseg * L], f32, tag="gate_ps")
                    psums[(g, hb)] = ps
                    sl = slice(hb * nseg, (hb + 1) * nseg)
                    if ILV:
                        # interleaved out: col (seg,l) -> addr l*2+seg, so one
                        # bn_stats with a non-collapsible strided AP covers
                        # both segments
                        pov = ps[:].rearrange("p (l s) -> p s l", s=nseg)
                    else:
                        pov = ps[:].rearrange("p (s l) -> p s l", s=nseg)
                    for j, off in enumerate((0, 2, 4)):     # x pairs {0,1},{2,3},{4}
                        nc.tensor.matmul(
                            pov, wx[:, (g * 3 + j) * 128:(g * 3 + j + 1) * 128],
                            xv[:, sl, off:off + L],
                            start=(j == 0), stop=False)
                    for k in range(ABL_HSHIFTS):             # h shifts
                        nc.tensor.matmul(
                            pov, wh[:, (g * 5 + k) * 128:(g * 5 + k + 1) * 128],
                            hv[:, sl, k:k + L],
                            start=False, stop=(k == ABL_HSHIFTS - 1))
                    if ILV:
                        nc.vector.bn_stats(
                            out=bnall[:, g * 2 + hb, :, :],
                            in_=pov)
                    else:
                        for bi in range(nseg)[:ABL_NBN]:
                            b = hb * nseg + bi
                            nc.vector.bn_stats(
                                out=bnall[:, g * 2 + b // 2, b % 2, :],
                                in_=ps[:, bi * L:(bi + 1) * L])

            # ---- GroupNorm stats: group A = gates {i,f,g} (overlaps o's
            # matmul phase), group B = {o} (short recurrent tail) ----
            s_sb = stp.tile([128, 16], f32, tag="s_sb")
            t_sb = stp.tile([128, 16], f32, tag="t_sb")

            def stats_chain(g0, ng, suf):
                NC_ = 4 * ng            # combos in this group
                wpad = ((4 * NC_ + 31) // 32) * 32 if TRREDUCE else 4 * NC_
                if TRREDUCE:
                    stage = stages[(suf, t % 2)]
                else:
                    stage = stp.tile([128, wpad], f32, tag=f"stage{suf}",
                                     name=f"stage{suf}")
                shf = stp.tile([128, wpad], f32, tag=f"shf{suf}",
                               name=f"shf{suf}")
                tmp = stp.tile([128, 2 * NC_], f32, tag=f"tmp{suf}",
                               name=f"tmp{suf}")
                bsl = bnall[:, 2 * g0:2 * (g0 + ng), :, :]
                means = bsl[:, :, :, 1::3]          # [128, 2ng, 2, 2]
                m2s = bsl[:, :, :, 2::3]
                b4h_g = cst[:, 48 + 8 * g0:48 + 8 * (g0 + ng)]
                nc.vector.tensor_tensor(out=stage[:, 0:2 * NC_], in0=means,
                                        in1=b4h_g, op=Alu.add)            # m'
                nc.vector.tensor_tensor(out=tmp[:], in0=stage[:, 0:2 * NC_],
                                        in1=stage[:, 0:2 * NC_], op=Alu.mult)
                nc.vector.scalar_tensor_tensor(
                    out=stage[:, 2 * NC_:4 * NC_], in0=m2s, scalar=1.0 / 128,
                    in1=tmp[:], op0=Alu.mult, op1=Alu.add)                # q
                if TRREDUCE:
                    # cross-partition group sums via 32x32 transpose-reduce:
                    # r[32a+i] = sum_{p in grp a} stage[p, blk*32+i]; then
                    # broadcast r across the block and transpose back.
                    nblk = wpad // 32
                    rsum = stp.tile([128, nblk], f32, tag=f"rsum{suf}",
                                    name=f"rsum{suf}")
                    for blk in range(nblk):
                        nc.vector.tensor_reduce(
                            out=rsum[:, blk:blk + 1],
                            in_=stage[:, blk * 32:(blk + 1) * 32],
                            axis=mybir.AxisListType.X, op=Alu.add,
                            apply_transpose=True)
                        nc.vector.tensor_scalar(
                            out=shf[:, blk * 32:(blk + 1) * 32], in0=ones32[:],
                            scalar1=rsum[:, blk:blk + 1], scalar2=None,
                            op0=Alu.mult)
                        nc.vector.transpose(
                            out=stage[:, blk * 32:(blk + 1) * 32],
                            in_=shf[:, blk * 32:(blk + 1) * 32])
                else:
                    for r in (16, 8, 4, 2, 1)[:ABL_BFLY]:
                        nc.vector.stream_shuffle(shf[:], stage[:],
                                                 [i ^ r for i in range(32)])
                        nc.vector.tensor_tensor(out=stage[:], in0=stage[:],
                                                in1=shf[:], op=Alu.add)
                sums = stp.tile([128, 2 * NC_], f32, tag=f"sums{suf}",
                                name=f"sums{suf}")
                st3 = stage[:, 0:4 * NC_].rearrange("p (c h) -> p c h", h=2)
                nc.vector.tensor_tensor(out=sums[:], in0=st3[:, :, 0],
                                        in1=st3[:, :, 1], op=Alu.add)
                s_m, s_q = sums[:, 0:NC_], sums[:, NC_:2 * NC_]
                csl = slice(4 * g0, 4 * (g0 + ng))
                # veps = S_q/64 - S_m^2/4096 + eps
                t2 = stp.tile([128, NC_], f32, tag=f"t2{suf}", name=f"t2{suf}")
                nc.vector.tensor_tensor(out=t2[:], in0=s_m, in1=s_m, op=Alu.mult)
                nc.vector.tensor_scalar(out=t2[:], in0=t2[:], scalar1=-1.0 / 4096,
                                        scalar2=EPS, op0=Alu.mult, op1=Alu.add)
                veps = stp.tile([128, NC_], f32, tag=f"veps{suf}",
                                name=f"veps{suf}")
                nc.vector.scalar_tensor_tensor(out=veps[:], in0=s_q,
                                               scalar=1.0 / 64, in1=t2[:],
                                               op0=Alu.mult, op1=Alu.add)
                # rs = rsqrt(veps): Quake seed + 2 Newton iterations
                rs = s_sb[:, csl]
                un = stp.tile([128, NC_], f32, tag=f"un{suf}", name=f"un{suf}")
                nc.vector.tensor_scalar(out=rs.bitcast(i32),
                                        in0=veps[:].bitcast(i32), scalar1=1,
                                        scalar2=None, op0=Alu.logical_shift_right)
                nc.vector.tensor_scalar(out=rs.bitcast(i32), in0=rs.bitcast(i32),
                                        scalar1=-1, scalar2=None,
                                        op0=Alu.bitwise_xor)
                nc.vector.tensor_scalar(out=rs.bitcast(i32), in0=rs.bitcast(i32),
                                        scalar1=0x5F3759E0, scalar2=None,
                                        op0=Alu.add)
                for _ in range(NEWTON_ITERS):
                    nc.vector.tensor_tensor(out=un[:], in0=veps[:], in1=rs,
                                            op=Alu.mult)
                    nc.vector.tensor_tensor(out=un[:], in0=un[:], in1=rs,
                                            op=Alu.mult)
                    nc.vector.tensor_scalar(out=un[:], in0=un[:], scalar1=-0.5,
                                            scalar2=1.5, op0=Alu.mult,
                                            op1=Alu.add)
                    nc.vector.tensor_tensor(out=rs, in0=rs, in1=un[:],
                                            op=Alu.mult)
                # s = rs*gw (skipped when gn_w==1) ; t = (cb - S_m/64)*s + gb
                if not TRIVIAL_GN:
                    nc.vector.tensor_tensor(out=rs, in0=rs, in1=cst[:, csl],
                                            op=Alu.mult)
                tg = t_sb[:, csl]
                nc.vector.scalar_tensor_tensor(out=tg, in0=s_m, scalar=-1.0 / 64,
                                               in1=cst[:, 16:32][:, csl],
                                               op0=Alu.mult, op1=Alu.add)
                nc.vector.tensor_tensor(out=tg, in0=tg, in1=rs, op=Alu.mult)
                if not TRIVIAL_GN:
                    nc.vector.tensor_tensor(out=tg, in0=tg,
                                            in1=cst[:, 32:48][:, csl], op=Alu.add)

            def norm(g):
                for b in range(BC):
                    c = 4 * g + b
                    pin = (psums[(g, b // 2)][:].rearrange(
                               "p (l s) -> p s l", s=2)[:, b % 2, :] if ILV else
                           psums[(g, b // 2)][:, (b % 2) * L:(b % 2 + 1) * L])
                    nc.scalar.activation(
                        out=gsb[g][:, b * L:(b + 1) * L],
                        in_=pin,
                        func=GATE_FUNC[g],
                        bias=t_sb[:, c:c + 1], scale=s_sb[:, c:c + 1])

            if SPLIT_STATS:
                stats_chain(0, 3, "A")
            else:
                stats_chain(0, 4, "A")
            for g in (0, 1, 2):
                norm(g)
            # ---- LSTM combine (runs under the o-gate matmul/stats phase) ----
            eng = nc.gpsimd if LSTM_GPSIMD else nc.vector
            eng.tensor_tensor(out=gsb[2][:], in0=gsb[0][:], in1=gsb[2][:],
                              op=Alu.mult)                          # i*g
            eng.tensor_tensor(out=gsb[1][:], in0=gsb[1][:], in1=cx[:],
                              op=Alu.mult)                          # f*c
            eng.tensor_tensor(out=cx[:], in0=gsb[1][:], in1=gsb[2][:],
                              op=Alu.add)                           # c'
            for b in range(BC):     # tanh(c') per b, ahead of ACT-o in queue
                nc.scalar.activation(out=tcy[:, b * L:(b + 1) * L],
                                     in_=cx[:, b * L:(b + 1) * L], func=Act.Tanh)
            if SPLIT_STATS:
                stats_chain(3, 1, "B")
            norm(3)
            hv_data = hv[:, :, 2:2 + L]
            for b in range(BC):                     # h' = sig(o)*tanh(c'), per b
                nc.vector.tensor_tensor(            # pipelines with ACT o calls
                    out=hv_data[:, b, :],
                    in0=gsb[3][:, b * L:(b + 1) * L],
                    in1=tcy[:, b * L:(b + 1) * L],
                    op=Alu.mult)
            nc.sync.dma_start(out_d[t].rearrange("b p l -> p b l"), hv_data)

        nc.sync.dma_start(cy_d.rearrange("b p l -> p b l"),
                          cx[:].rearrange("p (b l) -> p b l", b=BC))

    _split_multi_waits(nc)
    return nc


def prep_shared(conv_w, conv_b, gn_w, gn_b):
    """Host packing of weights/constants (shared across cores)."""
    W = np.asarray(conv_w, dtype=np.float32)        # (512, 192, 5)
    cb = np.asarray(conv_b, dtype=np.float32)
    gw = np.asarray(gn_w, dtype=np.float32)
    gb = np.asarray(gn_b, dtype=np.float32)

    wx = np.zeros((128, 4, 3, 128), dtype=np.float32)
    wh = np.zeros((128, 4, 5, 128), dtype=np.float32)
    for g in range(4):
        Wg = W[g * 128:(g + 1) * 128]               # (128, 192, 5)
        Wxg = Wg[:, :F, :]                          # (128, 64, 5)
        Whg = Wg[:, F:, :]                          # (128, 128, 5)
        wx[0:64, g, 0, :] = Wxg[:, :, 0].T
        wx[64:128, g, 0, :] = Wxg[:, :, 1].T
        wx[0:64, g, 1, :] = Wxg[:, :, 2].T
        wx[64:128, g, 1, :] = Wxg[:, :, 3].T
        wx[0:64, g, 2, :] = Wxg[:, :, 4].T
        for k in range(5):
            wh[:, g, k, :] = Whg[:, :, k].T

    cst = np.zeros((128, 80), dtype=np.float32)
    for g in range(4):
        for b in range(BC):
            cst[:, 0 + 4 * g + b] = gw[g * 128:(g + 1) * 128]
            cst[:, 16 + 4 * g + b] = cb[g * 128:(g + 1) * 128]
            cst[:, 32 + 4 * g + b] = gb[g * 128:(g + 1) * 128]
            for h in range(2):
                cst[:, 48 + (4 * g + b) * 2 + h] = cb[g * 128:(g + 1) * 128]
    return (wx.reshape(128, -1), wh.reshape(128, -1), cst)


_CACHE = {}


def kernel(inputs, conv_w, conv_b, gn_w, gn_b):
    x = np.asarray(inputs, dtype=np.float32)        # (32, 32, 64, 256)
    wx, wh, cst = prep_shared(conv_w, conv_b, gn_w, gn_b)

    trivial = bool(np.all(np.asarray(gn_w) == 1.0) and np.all(np.asarray(gn_b) == 0.0))
    key = ("nc", trivial)
    if key not in _CACHE:
        _CACHE[key] = build(SEQ, TRIVIAL_GN=trivial, TRREDUCE=True)
    nc = _CACHE[key]

    in_maps = []
    for c in range(NCORES):
        shard = np.ascontiguousarray(x[:, c * BC:(c + 1) * BC])
        in_maps.append({"x": shard, "wx": wx, "wh": wh, "cst": cst,
                        "zf": np.zeros((128, BC * SEG), np.float32)})

    res = run_bass_kernel_spmd(nc, in_maps, list(range(NCORES)))

    outputs = np.concatenate([res.results[c]["outs"] for c in range(NCORES)], axis=1)
    cy = np.concatenate([res.results[c]["cy"] for c in range(NCORES)], axis=0)
    hy = outputs[-1]
    return outputs, (hy, cy)


# revision 21
# speedup vs baseline: 1.0284x; 1.0141x over previous
"""ConvLSTM cell kernel for Trainium2 (8 NeuronCores, data-parallel over batch).

Problem: SEQ=32 steps of x_t (B,64,256); per step:
  gates = conv1d(concat([x_t, h], ch), W, k=5, pad=2) + b   -> (B,512,256)
  gates = GroupNorm(16 groups, eps=1e-5) * gn_w + gn_b
  i,f,g,o = split(gates); c = sig(f)*c + sig(i)*tanh(g); h = sig(o)*tanh(c)
Returns (outputs[SEQ,B,128,256], (h, c)).

Design (per core, 4 batch elems):
- conv as 8 accumulating PE matmuls per (gate, half-batch): K-dim packs two
  x-shifts per matmul via a double-loaded x tile with a 1-column offset
  between row halves; h-part is 5 full-K matmuls over a shared padded tile.
  All matmuls in float32r (full PE rate at N=512).
- GroupNorm stats: bn_stats per PSUM gate tile; cross-partition (32-ch group)
  reduction via a 3-op 32x32 transpose-reduce (tensor_reduce apply_transpose,
  broadcast, stream-transpose back); rsqrt via Quake-seed + 2 Newton
  iterations on DVE; result folded with conv bias, gn_w/gn_b into
  per-partition scale/bias consumed directly by the fused ScalarE
  sigmoid/tanh activation reading PSUM. Stats run as two pipelines:
  gates {i,f,g} overlap the o-gate matmul phase; only the o-gate chain sits
  on the recurrent tail, and the final h'=sig(o)*tanh(c') multiply is
  emitted per batch element to pipeline with the o-gate activations.
"""
import sys

sys.path.insert(0, "/opt/trn_rl_repo")

from contextlib import ExitStack

import numpy as np

import concourse.bass as bass
import concourse.tile as tile
from concourse import mybir
from concourse.bass_utils import run_bass_kernel_spmd

SEQ, B, F, L = 32, 32, 64, 256
OC = 128
NCORES = 8
BC = B // NCORES      # batch per core = 4
SEG = L + 4           # padded segment width = 260
EPS = 1e-5

f32 = mybir.dt.float32
f32r = mybir.dt.float32r
i32 = mybir.dt.int32
Alu = mybir.AluOpType
Act = mybir.ActivationFunctionType

GATE_ORDER = [0, 1, 2, 3]       # i, f, g, o  (o last: shortest recurrent tail)
GATE_FUNC = {0: Act.Sigmoid, 1: Act.Sigmoid, 2: Act.Tanh, 3: Act.Sigmoid}


def _split_multi_waits(nc):
    """This walrus build rejects instructions with >1 sem wait. Split extras
    onto preceding same-engine NoOps."""
    for fn in nc.m.functions:
        for bb in fn.blocks:
            new_insts = []
            for inst in bb.instructions:
                si = inst.sync_info
                waits = list(si.on_wait) if (si is not None and si.on_wait) else []
                if len(waits) > 1:
                    head, rest = waits[:-1], waits[-1:]
                    for j, w in enumerate(head):
                        nop = mybir.InstNoOp(name=f"{inst.name}-wsplit{j}", hint=None)
                        nop.engine = inst.engine
                        nop.sync_info = mybir.SyncInfo(on_wait=[w], on_update=[])
                        nc.register_instruction(nop)
                        new_insts.append(nop)
                    si.on_wait = rest
                new_insts.append(inst)
            bb.instructions = new_insts


def build(T=SEQ, TRIVIAL_GN=False, SPLIT_STATS=True, LSTM_GPSIMD=False,
          NEWTON_ITERS=2, STP_BUFS=2, N1024=False, TRREDUCE=False,
          ABL_HSHIFTS=5, ABL_BFLY=5, ABL_ACT8=False, ABL_NBN=2):
    nc = bass.Bass("TRN2", debug=False)
    x_d = nc.dram_tensor("x", [T, BC, F, L], f32r, kind="ExternalInput").ap()
    wx_d = nc.dram_tensor("wx", [128, 4 * 3 * 128], f32r, kind="ExternalInput").ap()
    wh_d = nc.dram_tensor("wh", [128, 4 * 5 * 128], f32r, kind="ExternalInput").ap()
    cst_d = nc.dram_tensor("cst", [128, 80], f32, kind="ExternalInput").ap()
    zf_d = nc.dram_tensor("zf", [128, BC * SEG], f32r, kind="ExternalInput").ap()
    out_d = nc.dram_tensor("outs", [T, BC, OC, L], f32r, kind="ExternalOutput").ap()
    cy_d = nc.dram_tensor("cy", [BC, OC, L], f32, kind="ExternalOutput").ap()

    with tile.TileContext(nc) as tc, ExitStack() as ctx:
        singles = ctx.enter_context(tc.tile_pool(name="singles", bufs=1))
        gps = ctx.enter_context(
            tc.tile_pool(name="gps", bufs=(4 if N1024 else 8), space="PSUM"))
        stp = ctx.enter_context(tc.tile_pool(name="stp", bufs=STP_BUFS))

        # ---- constants / weights ----
        wx = singles.tile([128, 4 * 3 * 128], f32r)     # lhsT packs: (gate, j, 128)
        wh = singles.tile([128, 4 * 5 * 128], f32r)     # lhsT packs: (gate, k, 128)
        cst = singles.tile([128, 80], f32)
        nc.sync.dma_start(wx[:], wx_d)
        nc.sync.dma_start(wh[:], wh_d)
        nc.sync.dma_start(cst[:], cst_d)
        gw4 = cst[:, 0:16]        # gn_w per (gate, b)
        b4 = cst[:, 16:32]        # conv_b per (gate, b)
        gb4 = cst[:, 32:48]       # gn_b per (gate, b)
        b4h = cst[:, 48:80]       # conv_b per (gate, b, half)

        # ---- state tiles ----
        xt = [singles.tile([128, BC * SEG], f32r, tag=f"xt{p}", name=f"xt{p}")
              for p in range(2)]
        ht = singles.tile([128, BC * SEG], f32r)
        cx = singles.tile([128, BC * L], f32)
        gsb = [singles.tile([128, BC * L], f32, tag=f"gsb{g}", name=f"gsb{g}")
               for g in range(4)]
        tcy = singles.tile([128, BC * L], f32)
        bnall = singles.tile([128, 8, 2, 6], f32)     # (gate*2+hb, b_in_half, sextet)
        ones32 = singles.tile([128, 32], f32)
        nc.vector.memset(ones32[:], 1.0)
        stages = {}
        if TRREDUCE:
            for suf, ng in (("A", 3 if SPLIT_STATS else 4), ("B", 1)):
                wpad = ((16 * ng + 31) // 32) * 32
                for par in range(2):
                    st_ = singles.tile([128, wpad], f32, tag=f"stg{suf}{par}",
                                       name=f"stg{suf}{par}")
                    nc.vector.memset(st_[:], 0.0)
                    stages[(suf, par)] = st_
        for t_ in xt:
            nc.sync.dma_start(t_[:], zf_d)
        nc.sync.dma_start(ht[:], zf_d)
        nc.vector.memset(cx[:], 0.0)

        def x_dma(t):
            src = x_d[t].rearrange("b c l -> c b l")
            dst = xt[t % 2][:].rearrange("p (s c) -> p s c", s=BC)
            nc.sync.dma_start(dst[0:F, :, 2:2 + L], src)
            nc.sync.dma_start(dst[F:2 * F, :, 1:1 + L], src)

        x_dma(0)

        for t in range(T):
            if t + 1 < T:
                x_dma(t + 1)
            xv = xt[t % 2][:].rearrange("p (s c) -> p s c", s=BC)
            hv = ht[:].rearrange("p (s c) -> p s c", s=BC)

            # ---- PE: 8 accumulating matmuls per (gate, half-batch)
            # (or per gate with N=1024 when N1024) ----
            psums = {}
            nhb = 1 if N1024 else 2
            nseg = BC // nhb
            for g in GATE_ORDER:
                for hb in range(nhb):
                    ps = gps.tile([128, nseg * L], f32, tag="gate_ps")
                    psums[(g, hb)] = ps
                    sl = slice(hb * nseg, (hb + 1) * nseg)
                    for j, off in enumerate((0, 2, 4)):     # x pairs {0,1},{2,3},{4}
                        nc.tensor.matmul(
                            ps[:], wx[:, (g * 3 + j) * 128:(g * 3 + j + 1) * 128],
                            xv[:, sl, off:off + L],
                            start=(j == 0), stop=False)
                    for k in range(ABL_HSHIFTS):             # h shifts
                        nc.tensor.matmul(
                            ps[:], wh[:, (g * 5 + k) * 128:(g * 5 + k + 1) * 128],
                            hv[:, sl, k:k + L],
                            start=False, stop=(k == ABL_HSHIFTS - 1))
                    for bi in range(nseg)[:ABL_NBN]:
                        b = hb * nseg + bi
                        nc.vector.bn_stats(
                            out=bnall[:, g * 2 + b // 2, b % 2, :],
                            in_=ps[:, bi * L:(bi + 1) * L])

            # ---- GroupNorm stats: group A = gates {i,f,g} (overlaps o's
            # matmul phase), group B = {o} (short recurrent tail) ----
            s_sb = stp.tile([128, 16], f32, tag="s_sb")
            t_sb = stp.tile([128, 16], f32, tag="t_sb")

            def stats_chain(g0, ng, suf):
                NC_ = 4 * ng            # combos in this group
                wpad = ((4 * NC_ + 31) // 32) * 32 if TRREDUCE else 4 * NC_
                if TRREDUCE:
                    stage = stages[(suf, t % 2)]
                else:
                    stage = stp.tile([128, wpad], f32, tag=f"stage{suf}",
                                     name=f"stage{suf}")
                shf = stp.tile([128, wpad], f32, tag=f"shf{suf}",
                               name=f"shf{suf}")
                tmp = stp.tile([128, 2 * NC_], f32, tag=f"tmp{suf}",
                               name=f"tmp{suf}")
                bsl = bnall[:, 2 * g0:2 * (g0 + ng), :, :]
                means = bsl[:, :, :, 1::3]          # [128, 2ng, 2, 2]
                m2s = bsl[:, :, :, 2::3]
                b4h_g = cst[:, 48 + 8 * g0:48 + 8 * (g0 + ng)]
                nc.vector.tensor_tensor(out=stage[:, 0:2 * NC_], in0=means,
                                        in1=b4h_g, op=Alu.add)            # m'
                nc.vector.tensor_tensor(out=tmp[:], in0=stage[:, 0:2 * NC_],
                                        in1=stage[:, 0:2 * NC_], op=Alu.mult)
                nc.vector.scalar_tensor_tensor(
                    out=stage[:, 2 * NC_:4 * NC_], in0=m2s, scalar=1.0 / 128,
                    in1=tmp[:], op0=Alu.mult, op1=Alu.add)                # q
                if TRREDUCE:
                    # cross-partition group sums via 32x32 transpose-reduce:
                    # r[32a+i] = sum_{p in grp a} stage[p, blk*32+i]; then
                    # broadcast r across the block and transpose back.
                    nblk = wpad // 32
                    rsum = stp.tile([128, nblk], f32, tag=f"rsum{suf}",
                                    name=f"rsum{suf}")
                    for blk in range(nblk):
                        nc.vector.tensor_reduce(
                            out=rsum[:, blk:blk + 1],
                            in_=stage[:, blk * 32:(blk + 1) * 32],
                            axis=mybir.AxisListType.X, op=Alu.add,
                            apply_transpose=True)
                        nc.vector.tensor_scalar(
                            out=shf[:, blk * 32:(blk + 1) * 32], in0=ones32[:],
                            scalar1=rsum[:, blk:blk + 1], scalar2=None,
                            op0=Alu.mult)
                        nc.vector.transpose(
                            out=stage[:, blk * 32:(blk + 1) * 32],
                            in_=shf[:, blk * 32:(blk + 1) * 32])
                else:
                    for r in (16, 8, 4, 2, 1)[:ABL_BFLY]:
                        nc.vector.stream_shuffle(shf[:], stage[:],
                                                 [i ^ r for i in range(32)])
                        nc.vector.tensor_tensor(out=stage[:], in0=stage[:],
                                                in1=shf[:], op=Alu.add)
                sums = stp.tile([128, 2 * NC_], f32, tag=f"sums{suf}",
                                name=f"sums{suf}")
                st3 = stage[:, 0:4 * NC_].rearrange("p (c h) -> p c h", h=2)
                nc.vector.tensor_tensor(out=sums[:], in0=st3[:, :, 0],
                                        in1=st3[:, :, 1], op=Alu.add)
                s_m, s_q = sums[:, 0:NC_], sums[:, NC_:2 * NC_]
                csl = slice(4 * g0, 4 * (g0 + ng))
                # veps = S_q/64 - S_m^2/4096 + eps
                t2 = stp.tile([128, NC_], f32, tag=f"t2{suf}", name=f"t2{suf}")
                nc.vector.tensor_tensor(out=t2[:], in0=s_m, in1=s_m, op=Alu.mult)
                nc.vector.tensor_scalar(out=t2[:], in0=t2[:], scalar1=-1.0 / 4096,
                                        scalar2=EPS, op0=Alu.mult, op1=Alu.add)
                veps = stp.tile([128, NC_], f32, tag=f"veps{suf}",
                                name=f"veps{suf}")
                nc.vector.scalar_tensor_tensor(out=veps[:], in0=s_q,
                                               scalar=1.0 / 64, in1=t2[:],
                                               op0=Alu.mult, op1=Alu.add)
                # rs = rsqrt(veps): Quake seed + 2 Newton iterations
                rs = s_sb[:, csl]
                un = stp.tile([128, NC_], f32, tag=f"un{suf}", name=f"un{suf}")
                nc.vector.tensor_scalar(out=rs.bitcast(i32),
                                        in0=veps[:].bitcast(i32), scalar1=1,
                                        scalar2=None, op0=Alu.logical_shift_right)
                nc.vector.tensor_scalar(out=rs.bitcast(i32), in0=rs.bitcast(i32),
                                        scalar1=-1, scalar2=None,
                                        op0=Alu.bitwise_xor)
                nc.vector.tensor_scalar(out=rs.bitcast(i32), in0=rs.bitcast(i32),
                                        scalar1=0x5F3759E0, scalar2=None,
                                        op0=Alu.add)
                for _ in range(NEWTON_ITERS):
                    # y' = (1.5 - 0.5*v*y^2) * y, re-associated into 3 ops:
                    nc.vector.tensor_tensor(out=un[:], in0=veps[:], in1=rs,
                                            op=Alu.mult)            # v*y
                    nc.vector.scalar_tensor_tensor(
                        out=un[:], in0=un[:], scalar=-0.5, in1=rs,
                        op0=Alu.mult, op1=Alu.mult)                 # -0.5*v*y^2
                    nc.vector.scalar_tensor_tensor(
                        out=rs, in0=un[:], scalar=1.5, in1=rs,
                        op0=Alu.add, op1=Alu.mult)                  # (..+1.5)*y
                # s = rs*gw (skipped when gn_w==1) ; t = (cb - S_m/64)*s + gb
                if not TRIVIAL_GN:
                    nc.vector.tensor_tensor(out=rs, in0=rs, in1=cst[:, csl],
                                            op=Alu.mult)
                tg = t_sb[:, csl]
                nc.vector.scalar_tensor_tensor(out=tg, in0=s_m, scalar=-1.0 / 64,
                                               in1=cst[:, 16:32][:, csl],
                                               op0=Alu.mult, op1=Alu.add)
                nc.vector.tensor_tensor(out=tg, in0=tg, in1=rs, op=Alu.mult)
                if not TRIVIAL_GN:
                    nc.vector.tensor_tensor(out=tg, in0=tg,
                                            in1=cst[:, 32:48][:, csl], op=Alu.add)

            def norm(g):
                for b in range(BC):
                    c = 4 * g + b
                    nc.scalar.activation(
                        out=gsb[g][:, b * L:(b + 1) * L],
                        in_=psums[(g, b // 2)][:, (b % 2) * L:(b % 2 + 1) * L],
                        func=GATE_FUNC[g],
                        bias=t_sb[:, c:c + 1], scale=s_sb[:, c:c + 1])

            if SPLIT_STATS:
                stats_chain(0, 3, "A")
            else:
                stats_chain(0, 4, "A")
            for g in (0, 1, 2):
                norm(g)
            # ---- LSTM combine (runs under the o-gate matmul/stats phase) ----
            eng = nc.gpsimd if LSTM_GPSIMD else nc.vector
            eng.tensor_tensor(out=gsb[2][:], in0=gsb[0][:], in1=gsb[2][:],
                              op=Alu.mult)                          # i*g
            eng.tensor_tensor(out=gsb[1][:], in0=gsb[1][:], in1=cx[:],
                              op=Alu.mult)                          # f*c
            eng.tensor_tensor(out=cx[:], in0=gsb[1][:], in1=gsb[2][:],
                              op=Alu.add)                           # c'
            for b in range(BC):     # tanh(c') per b, ahead of ACT-o in queue
                nc.scalar.activation(out=tcy[:, b * L:(b + 1) * L],
                                     in_=cx[:, b * L:(b + 1) * L], func=Act.Tanh)
            if SPLIT_STATS:
                stats_chain(3, 1, "B")
            norm(3)
            hv_data = hv[:, :, 2:2 + L]
            for b in range(BC):                     # h' = sig(o)*tanh(c'), per b
                nc.vector.tensor_tensor(            # pipelines with ACT o calls
                    out=hv_data[:, b, :],
                    in0=gsb[3][:, b * L:(b + 1) * L],
                    in1=tcy[:, b * L:(b + 1) * L],
                    op=Alu.mult)
            nc.sync.dma_start(out_d[t].rearrange("b p l -> p b l"), hv_data)

        nc.sync.dma_start(cy_d.rearrange("b p l -> p b l"),
                          cx[:].rearrange("p (b l) -> p b l", b=BC))

    _split_multi_waits(nc)
    return nc


def prep_shared(conv_w, conv_b, gn_w, gn_b):
    """Host packing of weights/constants (shared across cores)."""
    W = np.asarray(conv_w, dtype=np.float32)        # (512, 192, 5)
    cb = np.asarray(conv_b, dtype=np.float32)
    gw = np.asarray(gn_w, dtype=np.float32)
    gb = np.asarray(gn_b, dtype=np.float32)

    wx = np.zeros((128, 4, 3, 128), dtype=np.float32)
    wh = np.zeros((128, 4, 5, 128), dtype=np.float32)
    for g in range(4):
        Wg = W[g * 128:(g + 1) * 128]               # (128, 192, 5)
        Wxg = Wg[:, :F, :]                          # (128, 64, 5)
        Whg = Wg[:, F:, :]                          # (128, 128, 5)
        wx[0:64, g, 0, :] = Wxg[:, :, 0].T
        wx[64:128, g, 0, :] = Wxg[:, :, 1].T
        wx[0:64, g, 1, :] = Wxg[:, :, 2].T
        wx[64:128, g, 1, :] = Wxg[:, :, 3].T
        wx[0:64, g, 2, :] = Wxg[:, :, 4].T
        for k in range(5):
            wh[:, g, k, :] = Whg[:, :, k].T

    cst = np.zeros((128, 80), dtype=np.float32)
    for g in range(4):
        for b in range(BC):
            cst[:, 0 + 4 * g + b] = gw[g * 128:(g + 1) * 128]
            cst[:, 16 + 4 * g + b] = cb[g * 128:(g + 1) * 128]
            cst[:, 32 + 4 * g + b] = gb[g * 128:(g + 1) * 128]
            for h in range(2):
                cst[:, 48 + (4 * g + b) * 2 + h] = cb[g * 128:(g + 1) * 128]
    return (wx.reshape(128, -1), wh.reshape(128, -1), cst)


_CACHE = {}


def kernel(inputs, conv_w, conv_b, gn_w, gn_b):
    x = np.asarray(inputs, dtype=np.float32)        # (32, 32, 64, 256)
    wx, wh, cst = prep_shared(conv_w, conv_b, gn_w, gn_b)

    trivial = bool(np.all(np.asarray(gn_w) == 1.0) and np.all(np.asarray(gn_b) == 0.0))
    key = ("nc", trivial)
    if key not in _CACHE:
        _CACHE[key] = build(SEQ, TRIVIAL_GN=trivial, TRREDUCE=True)
    nc = _CACHE[key]

    in_maps = []
    for c in range(NCORES):
        shard = np.ascontiguousarray(x[:, c * BC:(c + 1) * BC])
        in_maps.append({"x": shard, "wx": wx, "wh": wh, "cst": cst,
                        "zf": np.zeros((128, BC * SEG), np.float32)})

    res = run_bass_kernel_spmd(nc, in_maps, list(range(NCORES)))

    outputs = np.concatenate([res.results[c]["outs"] for c in range(NCORES)], axis=1)
    cy = np.concatenate([res.results[c]["cy"] for c in range(NCORES)], axis=0)
    hy = outputs[-1]
    return outputs, (hy, cy)


# revision 22
# speedup vs baseline: 1.0357x; 1.0071x over previous
"""ConvLSTM cell kernel for Trainium2 (8 NeuronCores, data-parallel over batch).

Problem: SEQ=32 steps of x_t (B,64,256); per step:
  gates = conv1d(concat([x_t, h], ch), W, k=5, pad=2) + b   -> (B,512,256)
  gates = GroupNorm(16 groups, eps=1e-5) * gn_w + gn_b
  i,f,g,o = split(gates); c = sig(f)*c + sig(i)*tanh(g); h = sig(o)*tanh(c)
Returns (outputs[SEQ,B,128,256], (h, c)).

Design (per core, 4 batch elems):
- conv as 8 accumulating PE matmuls per (gate, half-batch): K-dim packs two
  x-shifts per matmul via a double-loaded x tile with a 1-column offset
  between row halves; h-part is 5 full-K matmuls over a shared padded tile.
  All matmuls in float32r (full PE rate at N=512).
- GroupNorm stats: bn_stats per PSUM gate tile; cross-partition (32-ch group)
  reduction via a 3-op 32x32 transpose-reduce (tensor_reduce apply_transpose,
  broadcast, stream-transpose back); rsqrt via Quake-seed + 2 Newton
  iterations on DVE; result folded with conv bias, gn_w/gn_b into
  per-partition scale/bias consumed directly by the fused ScalarE
  sigmoid/tanh activation reading PSUM. Stats run as two pipelines:
  gates {i,f,g} overlap the o-gate matmul phase; only the o-gate chain sits
  on the recurrent tail, and the final h'=sig(o)*tanh(c') multiply is
  emitted per batch element to pipeline with the o-gate activations.
"""
import sys

sys.path.insert(0, "/opt/trn_rl_repo")

from contextlib import ExitStack

import numpy as np

import concourse.bass as bass
import concourse.tile as tile
from concourse import mybir
from concourse.bass_utils import run_bass_kernel_spmd

SEQ, B, F, L = 32, 32, 64, 256
OC = 128
NCORES = 8
BC = B // NCORES      # batch per core = 4
SEG = L + 4           # padded segment width = 260
EPS = 1e-5

f32 = mybir.dt.float32
f32r = mybir.dt.float32r
i32 = mybir.dt.int32
Alu = mybir.AluOpType
Act = mybir.ActivationFunctionType

GATE_ORDER = [0, 1, 2, 3]       # i, f, g, o  (o last: shortest recurrent tail)
GATE_FUNC = {0: Act.Sigmoid, 1: Act.Sigmoid, 2: Act.Tanh, 3: Act.Sigmoid}


def _split_multi_waits(nc):
    """This walrus build rejects instructions with >1 sem wait. Split extras
    onto preceding same-engine NoOps."""
    for fn in nc.m.functions:
        for bb in fn.blocks:
            new_insts = []
            for inst in bb.instructions:
                si = inst.sync_info
                waits = list(si.on_wait) if (si is not None and si.on_wait) else []
                if len(waits) > 1:
                    head, rest = waits[:-1], waits[-1:]
                    for j, w in enumerate(head):
                        nop = mybir.InstNoOp(name=f"{inst.name}-wsplit{j}", hint=None)
                        nop.engine = inst.engine
                        nop.sync_info = mybir.SyncInfo(on_wait=[w], on_update=[])
                        nc.register_instruction(nop)
                        new_insts.append(nop)
                    si.on_wait = rest
                new_insts.append(inst)
            bb.instructions = new_insts


def build(T=SEQ, TRIVIAL_GN=False, SPLIT_STATS=True, LSTM_GPSIMD=False,
          NEWTON_ITERS=2, STP_BUFS=2, N1024=False, TRREDUCE=False,
          ABL_HSHIFTS=5, ABL_BFLY=5, ABL_ACT8=False, ABL_NBN=2):
    nc = bass.Bass("TRN2", debug=False)
    x_d = nc.dram_tensor("x", [T, BC, F, L], f32r, kind="ExternalInput").ap()
    wx_d = nc.dram_tensor("wx", [128, 4 * 3 * 128], f32r, kind="ExternalInput").ap()
    wh_d = nc.dram_tensor("wh", [128, 4 * 5 * 128], f32r, kind="ExternalInput").ap()
    cst_d = nc.dram_tensor("cst", [128, 80], f32, kind="ExternalInput").ap()
    zf_d = nc.dram_tensor("zf", [128, BC * SEG], f32r, kind="ExternalInput").ap()
    out_d = nc.dram_tensor("outs", [T, BC, OC, L], f32r, kind="ExternalOutput").ap()
    cy_d = nc.dram_tensor("cy", [BC, OC, L], f32, kind="ExternalOutput").ap()

    with tile.TileContext(nc) as tc, ExitStack() as ctx:
        singles = ctx.enter_context(tc.tile_pool(name="singles", bufs=1))
        gps = ctx.enter_context(
            tc.tile_pool(name="gps", bufs=(4 if N1024 else 8), space="PSUM"))
        stp = ctx.enter_context(tc.tile_pool(name="stp", bufs=STP_BUFS))

        # ---- constants / weights ----
        wx = singles.tile([128, 4 * 3 * 128], f32r)     # lhsT packs: (gate, j, 128)
        wh = singles.tile([128, 4 * 5 * 128], f32r)     # lhsT packs: (gate, k, 128)
        cst = singles.tile([128, 80], f32)
        nc.sync.dma_start(wx[:], wx_d)
        nc.sync.dma_start(wh[:], wh_d)
        nc.sync.dma_start(cst[:], cst_d)
        gw4 = cst[:, 0:16]        # gn_w per (gate, b)
        b4 = cst[:, 16:32]        # conv_b per (gate, b)
        gb4 = cst[:, 32:48]       # gn_b per (gate, b)
        b4h = cst[:, 48:80]       # conv_b per (gate, b, half)

        # ---- state tiles ----
        xt = [singles.tile([128, BC * SEG], f32r, tag=f"xt{p}", name=f"xt{p}")
              for p in range(2)]
        ht = singles.tile([128, BC * SEG], f32r)
        cx = singles.tile([128, BC * L], f32)
        gsb = [singles.tile([128, BC * L], f32, tag=f"gsb{g}", name=f"gsb{g}")
               for g in range(4)]
        tcy = singles.tile([128, BC * L], f32)
        bnall = singles.tile([128, 8, 2, 6], f32)     # (gate*2+hb, b_in_half, sextet)
        ones32 = singles.tile([128, 32], f32)
        nc.vector.memset(ones32[:], 1.0)
        stages = {}
        if TRREDUCE:
            for suf, ng in (("A", 3 if SPLIT_STATS else 4), ("B", 1)):
                wpad = ((16 * ng + 31) // 32) * 32
                for par in range(2):
                    st_ = singles.tile([128, wpad], f32, tag=f"stg{suf}{par}",
                                       name=f"stg{suf}{par}")
                    nc.vector.memset(st_[:], 0.0)
                    stages[(suf, par)] = st_
        for t_ in xt:
            nc.sync.dma_start(t_[:], zf_d)
        nc.sync.dma_start(ht[:], zf_d)
        nc.vector.memset(cx[:], 0.0)

        def x_dma(t):
            src = x_d[t].rearrange("b c l -> c b l")
            dst = xt[t % 2][:].rearrange("p (s c) -> p s c", s=BC)
            nc.sync.dma_start(dst[0:F, :, 2:2 + L], src)
            nc.sync.dma_start(dst[F:2 * F, :, 1:1 + L], src)

        x_dma(0)

        for t in range(T):
            if t + 1 < T:
                x_dma(t + 1)
            xv = xt[t % 2][:].rearrange("p (s c) -> p s c", s=BC)
            hv = ht[:].rearrange("p (s c) -> p s c", s=BC)

            # ---- PE: 8 accumulating matmuls per (gate, half-batch)
            # (or per gate with N=1024 when N1024) ----
            psums = {}
            nhb = 1 if N1024 else 2
            nseg = BC // nhb
            for g in GATE_ORDER:
                for hb in range(nhb):
                    ps = gps.tile([128, nseg * L], f32, tag="gate_ps")
                    psums[(g, hb)] = ps
                    sl = slice(hb * nseg, (hb + 1) * nseg)
                    for j, off in enumerate((0, 2, 4)):     # x pairs {0,1},{2,3},{4}
                        nc.tensor.matmul(
                            ps[:], wx[:, (g * 3 + j) * 128:(g * 3 + j + 1) * 128],
                            xv[:, sl, off:off + L],
                            start=(j == 0), stop=False)
                    for k in range(ABL_HSHIFTS):             # h shifts
                        nc.tensor.matmul(
                            ps[:], wh[:, (g * 5 + k) * 128:(g * 5 + k + 1) * 128],
                            hv[:, sl, k:k + L],
                            start=False, stop=(k == ABL_HSHIFTS - 1))
                    for bi in range(nseg)[:ABL_NBN]:
                        b = hb * nseg + bi
                        nc.vector.bn_stats(
                            out=bnall[:, g * 2 + b // 2, b % 2, :],
                            in_=ps[:, bi * L:(bi + 1) * L])

            # ---- GroupNorm stats: group A = gates {i,f,g} (overlaps o's
            # matmul phase), group B = {o} (short recurrent tail) ----
            s_sb = stp.tile([128, 16], f32, tag="s_sb")
            t_sb = stp.tile([128, 16], f32, tag="t_sb")

            def stats_chain(g0, ng, suf):
                NC_ = 4 * ng            # combos in this group
                wpad = ((4 * NC_ + 31) // 32) * 32 if TRREDUCE else 4 * NC_
                if TRREDUCE:
                    stage = stages[(suf, t % 2)]
                else:
                    stage = stp.tile([128, wpad], f32, tag=f"stage{suf}",
                                     name=f"stage{suf}")
                shf = stp.tile([128, wpad], f32, tag=f"shf{suf}",
                               name=f"shf{suf}")
                tmp = stp.tile([128, 2 * NC_], f32, tag=f"tmp{suf}",
                               name=f"tmp{suf}")
                bsl = bnall[:, 2 * g0:2 * (g0 + ng), :, :]
                means = bsl[:, :, :, 1::3]          # [128, 2ng, 2, 2]
                m2s = bsl[:, :, :, 2::3]
                b4h_g = cst[:, 48 + 8 * g0:48 + 8 * (g0 + ng)]
                nc.vector.tensor_tensor(out=stage[:, 0:2 * NC_], in0=means,
                                        in1=b4h_g, op=Alu.add)            # m'
                nc.vector.tensor_tensor(out=tmp[:], in0=stage[:, 0:2 * NC_],
                                        in1=stage[:, 0:2 * NC_], op=Alu.mult)
                nc.vector.scalar_tensor_tensor(
                    out=stage[:, 2 * NC_:4 * NC_], in0=m2s, scalar=1.0 / 128,
                    in1=tmp[:], op0=Alu.mult, op1=Alu.add)                # q
                if TRREDUCE:
                    # cross-partition group sums via 32x32 transpose-reduce:
                    # r[32a+i] = sum_{p in grp a} stage[p, blk*32+i]; then
                    # broadcast r across the block and transpose back.
                    nblk = wpad // 32
                    rsum = stp.tile([128, nblk], f32, tag=f"rsum{suf}",
                                    name=f"rsum{suf}")
                    for blk in range(nblk):
                        nc.vector.tensor_reduce(
                            out=rsum[:, blk:blk + 1],
                            in_=stage[:, blk * 32:(blk + 1) * 32],
                            axis=mybir.AxisListType.X, op=Alu.add,
                            apply_transpose=True)
                        nc.vector.tensor_scalar(
                            out=shf[:, blk * 32:(blk + 1) * 32], in0=ones32[:],
                            scalar1=rsum[:, blk:blk + 1], scalar2=None,
                            op0=Alu.mult)
                        nc.vector.transpose(
                            out=stage[:, blk * 32:(blk + 1) * 32],
                            in_=shf[:, blk * 32:(blk + 1) * 32])
                else:
                    for r in (16, 8, 4, 2, 1)[:ABL_BFLY]:
                        nc.vector.stream_shuffle(shf[:], stage[:],
                                                 [i ^ r for i in range(32)])
                        nc.vector.tensor_tensor(out=stage[:], in0=stage[:],
                                                in1=shf[:], op=Alu.add)
                sums = stp.tile([128, 2 * NC_], f32, tag=f"sums{suf}",
                                name=f"sums{suf}")
                st3 = stage[:, 0:4 * NC_].rearrange("p (c h) -> p c h", h=2)
                nc.vector.tensor_tensor(out=sums[:], in0=st3[:, :, 0],
                                        in1=st3[:, :, 1], op=Alu.add)
                s_m, s_q = sums[:, 0:NC_], sums[:, NC_:2 * NC_]
                csl = slice(4 * g0, 4 * (g0 + ng))
                # veps = S_q/64 - S_m^2/4096 + eps
                t2 = stp.tile([128, NC_], f32, tag=f"t2{suf}", name=f"t2{suf}")
                nc.vector.tensor_tensor(out=t2[:], in0=s_m, in1=s_m, op=Alu.mult)
                nc.vector.tensor_scalar(out=t2[:], in0=t2[:], scalar1=-1.0 / 4096,
                                        scalar2=EPS, op0=Alu.mult, op1=Alu.add)
                veps = stp.tile([128, NC_], f32, tag=f"veps{suf}",
                                name=f"veps{suf}")
                nc.vector.scalar_tensor_tensor(out=veps[:], in0=s_q,
                                               scalar=1.0 / 64, in1=t2[:],
                                               op0=Alu.mult, op1=Alu.add)
                # rs = rsqrt(veps): Quake seed + 2 Newton iterations
                rs = s_sb[:, csl]
                un = stp.tile([128, NC_], f32, tag=f"un{suf}", name=f"un{suf}")
                nc.vector.tensor_scalar(out=rs.bitcast(i32),
                                        in0=veps[:].bitcast(i32), scalar1=1,
                                        scalar2=-1,
                                        op0=Alu.logical_shift_right,
                                        op1=Alu.bitwise_xor)   # ~(v>>1)
                nc.vector.tensor_scalar(out=rs.bitcast(i32), in0=rs.bitcast(i32),
                                        scalar1=0x5F3759E0, scalar2=None,
                                        op0=Alu.add)
                for _ in range(NEWTON_ITERS):
                    # y' = (1.5 - 0.5*v*y^2) * y, re-associated into 3 ops:
                    nc.vector.tensor_tensor(out=un[:], in0=veps[:], in1=rs,
                                            op=Alu.mult)            # v*y
                    nc.vector.scalar_tensor_tensor(
                        out=un[:], in0=un[:], scalar=-0.5, in1=rs,
                        op0=Alu.mult, op1=Alu.mult)                 # -0.5*v*y^2
                    nc.vector.scalar_tensor_tensor(
                        out=rs, in0=un[:], scalar=1.5, in1=rs,
                        op0=Alu.add, op1=Alu.mult)                  # (..+1.5)*y
                # s = rs*gw (skipped when gn_w==1) ; t = (cb - S_m/64)*s + gb
                if not TRIVIAL_GN:
                    nc.vector.tensor_tensor(out=rs, in0=rs, in1=cst[:, csl],
                                            op=Alu.mult)
                tg = t_sb[:, csl]
                nc.vector.scalar_tensor_tensor(out=tg, in0=s_m, scalar=-1.0 / 64,
                                               in1=cst[:, 16:32][:, csl],
                                               op0=Alu.mult, op1=Alu.add)
                nc.vector.tensor_tensor(out=tg, in0=tg, in1=rs, op=Alu.mult)
                if not TRIVIAL_GN:
                    nc.vector.tensor_tensor(out=tg, in0=tg,
                                            in1=cst[:, 32:48][:, csl], op=Alu.add)

            def norm(g):
                for b in range(BC):
                    c = 4 * g + b
                    nc.scalar.activation(
                        out=gsb[g][:, b * L:(b + 1) * L],
                        in_=psums[(g, b // 2)][:, (b % 2) * L:(b % 2 + 1) * L],
                        func=GATE_FUNC[g],
                        bias=t_sb[:, c:c + 1], scale=s_sb[:, c:c + 1])

            if SPLIT_STATS:
                stats_chain(0, 3, "A")
            else:
                stats_chain(0, 4, "A")
            for g in (0, 1, 2):
                norm(g)
            # ---- LSTM combine (runs under the o-gate matmul/stats phase) ----
            eng = nc.gpsimd if LSTM_GPSIMD else nc.vector
            eng.tensor_tensor(out=gsb[2][:], in0=gsb[0][:], in1=gsb[2][:],
                              op=Alu.mult)                          # i*g
            eng.tensor_tensor(out=gsb[1][:], in0=gsb[1][:], in1=cx[:],
                              op=Alu.mult)                          # f*c
            eng.tensor_tensor(out=cx[:], in0=gsb[1][:], in1=gsb[2][:],
                              op=Alu.add)                           # c'
            for b in range(BC):     # tanh(c') per b, ahead of ACT-o in queue
                nc.scalar.activation(out=tcy[:, b * L:(b + 1) * L],
                                     in_=cx[:, b * L:(b + 1) * L], func=Act.Tanh)
            if SPLIT_STATS:
                stats_chain(3, 1, "B")
            norm(3)
            hv_data = hv[:, :, 2:2 + L]
            for b in range(BC):                     # h' = sig(o)*tanh(c'), per b
                nc.vector.tensor_tensor(            # pipelines with ACT o calls
                    out=hv_data[:, b, :],
                    in0=gsb[3][:, b * L:(b + 1) * L],
                    in1=tcy[:, b * L:(b + 1) * L],
                    op=Alu.mult)
            nc.sync.dma_start(out_d[t].rearrange("b p l -> p b l"), hv_data)

        nc.sync.dma_start(cy_d.rearrange("b p l -> p b l"),
                          cx[:].rearrange("p (b l) -> p b l", b=BC))

    _split_multi_waits(nc)
    return nc


def prep_shared(conv_w, conv_b, gn_w, gn_b):
    """Host packing of weights/constants (shared across cores)."""
    W = np.asarray(conv_w, dtype=np.float32)        # (512, 192, 5)
    cb = np.asarray(conv_b, dtype=np.float32)
    gw = np.asarray(gn_w, dtype=np.float32)
    gb = np.asarray(gn_b, dtype=np.float32)

    wx = np.zeros((128, 4, 3, 128), dtype=np.float32)
    wh = np.zeros((128, 4, 5, 128), dtype=np.float32)
    for g in range(4):
        Wg = W[g * 128:(g + 1) * 128]               # (128, 192, 5)
        Wxg = Wg[:, :F, :]                          # (128, 64, 5)
        Whg = Wg[:, F:, :]                          # (128, 128, 5)
        wx[0:64, g, 0, :] = Wxg[:, :, 0].T
        wx[64:128, g, 0, :] = Wxg[:, :, 1].T
        wx[0:64, g, 1, :] = Wxg[:, :, 2].T
        wx[64:128, g, 1, :] = Wxg[:, :, 3].T
        wx[0:64, g, 2, :] = Wxg[:, :, 4].T
        for k in range(5):
            wh[:, g, k, :] = Whg[:, :, k].T

    cst = np.zeros((128, 80), dtype=np.float32)
    for g in range(4):
        for b in range(BC):
            cst[:, 0 + 4 * g + b] = gw[g * 128:(g + 1) * 128]
            cst[:, 16 + 4 * g + b] = cb[g * 128:(g + 1) * 128]
            cst[:, 32 + 4 * g + b] = gb[g * 128:(g + 1) * 128]
            for h in range(2):
                cst[:, 48 + (4 * g + b) * 2 + h] = cb[g * 128:(g + 1) * 128]
    return (wx.reshape(128, -1), wh.reshape(128, -1), cst)


_CACHE = {}


def kernel(inputs, conv_w, conv_b, gn_w, gn_b):
    x = np.asarray(inputs, dtype=np.float32)        # (32, 32, 64, 256)
    wx, wh, cst = prep_shared(conv_w, conv_b, gn_w, gn_b)

    trivial = bool(np.all(np.asarray(gn_w) == 1.0) and np.all(np.asarray(gn_b) == 0.0))
    key = ("nc", trivial)
    if key not in _CACHE:
        _CACHE[key] = build(SEQ, TRIVIAL_GN=trivial, TRREDUCE=True)
    nc = _CACHE[key]

    in_maps = []
    for c in range(NCORES):
        shard = np.ascontiguousarray(x[:, c * BC:(c + 1) * BC])
        in_maps.append({"x": shard, "wx": wx, "wh": wh, "cst": cst,
                        "zf": np.zeros((128, BC * SEG), np.float32)})

    res = run_bass_kernel_spmd(nc, in_maps, list(range(NCORES)))

    outputs = np.concatenate([res.results[c]["outs"] for c in range(NCORES)], axis=1)
    cy = np.concatenate([res.results[c]["cy"] for c in range(NCORES)], axis=0)
    hy = outputs[-1]
    return outputs, (hy, cy)
